# revision 11
# baseline (speedup 1.0000x reference)
"""Trainium2 Bass kernel for causal GQA self-attention (dense transformer block).

Sharding: 8 cores = 2 batches x 4 kv-head groups. Each core computes, for its
(batch, kv-head) pair: Q/K/V projections for its 4 q-heads + 1 kv-head,
RMS-norm + partial RoPE + q_gain, causal softmax(QK^T)V, and its partial
output projection (out_w column block). Host sums the 4 partial outputs per
batch and transposes back.

On-chip layout is "transposed": activations live as [feature, token] so every
matmul has a 512-wide moving dim (fp32r full-rate) and softmax normalization
is done with PE ones-matmuls along the kt partition axis.
"""

import sys

sys.path.insert(0, "/opt/trn_rl_repo")

import contextlib

import numpy as np

import concourse.bass as bass
import concourse.tile as tile
from concourse import bacc, mybir
from concourse.bass_utils import run_bass_kernel_spmd
from concourse.masks import make_identity

F32 = mybir.dt.float32
F32R = mybir.dt.float32r

EPS = 1.1920929e-07


def build_core_kernel(nc, S, D, NH, HD, RD, O):
    """Emit the per-core program. NH q-heads (each HD wide), one kv head."""
    QT = 512                      # qt tile width (moving dim)
    KC = D // 128                 # contraction chunks for projections
    KH = KC // 2                  # x chunks per half-tile
    JT = S // QT                  # qt tiles
    KT = S // 128                 # kt tiles
    RPB = KT // JT                # kt(128) tiles per qt(512) tile = 4
    HC = NH * HD // 128           # head-dim chunks of the core's q block
    assert HD == 128 and QT == 512 and RPB == 4 and KC % 2 == 0

    dram = lambda name, shape, dt, kind: nc.dram_tensor(name, shape, dt, kind=kind).ap()
    xT = dram("xT", [D, S], F32R, "ExternalInput")
    qw = dram("qw", [D, NH * HD], F32R, "ExternalInput")
    kw = dram("kw", [D, HD], F32R, "ExternalInput")
    vw = dram("vw", [D, HD], F32R, "ExternalInput")
    ow = dram("ow", [NH * HD, O], F32R, "ExternalInput")
    cc = dram("cc", [RD, S], F32, "ExternalInput")
    ssn = dram("ss", [RD, S], F32, "ExternalInput")
    gain = dram("gain", [1, NH], F32, "ExternalInput")
    ones_kd = dram("ones_k", [128, 1], F32R, "ExternalInput")
    ones_pd = dram("ones_p", [1, 128], F32R, "ExternalInput")
    outT = dram("outT", [O, S], F32, "ExternalOutput")
    srow_d = nc.dram_tensor("srow_d", [NH + 1, S], F32R).ap()  # internal scratch

    scale = 1.0 / float(np.sqrt(HD))

    with tile.TileContext(nc) as tc, nc.allow_low_precision(
        reason="float32r tiles are 4-byte fp32"
    ), contextlib.ExitStack() as ctx:
        consts = ctx.enter_context(tc.tile_pool(name="consts", bufs=1))
        resid = ctx.enter_context(tc.tile_pool(name="resid", bufs=1))
        bigw = ctx.enter_context(tc.tile_pool(name="bigw", bufs=1))
        rows = ctx.enter_context(tc.tile_pool(name="rows", bufs=2))
        psA = ctx.enter_context(tc.tile_pool(name="psA", bufs=2, space="PSUM"))
        psM = ctx.enter_context(tc.tile_pool(name="psM", bufs=3, space="PSUM"))
        psR = ctx.enter_context(tc.tile_pool(name="psR", bufs=2, space="PSUM"))

        # constants
        ones_k = consts.tile([128, 1], F32R, tag="ones_k")
        nc.sync.dma_start(ones_k, ones_kd)
        ones_p = consts.tile([1, 128], F32R, tag="ones_p")
        nc.sync.dma_start(ones_p, ones_pd)
        ident = consts.tile([128, 128], F32, tag="ident")
        make_identity(nc, ident)
        cc_sb = consts.tile([RD, S], F32, tag="cc")
        nc.sync.dma_start(cc_sb, cc)
        ss_sb = consts.tile([RD, S], F32, tag="ss")
        nc.sync.dma_start(ss_sb, ssn)
        gain_sb = consts.tile([1, NH], F32, tag="gain")
        nc.sync.dma_start(gain_sb, gain)
        eps_sb = consts.tile([1, 1], F32, tag="eps")
        nc.vector.memset(eps_sb, EPS)

        # big weight buffer: q_w during phase 1, out_w during phase 3
        qw_sb = bigw.tile([128, KC, NH * HD], F32R, tag="bigw")
        nc.sync.dma_start(qw_sb, qw.rearrange("(c p) m -> p c m", p=128))

        # activation residents: [feature 128, token S]
        qT = [resid.tile([128, S], F32R, tag=f"qT{h}", name=f"qT{h}") for h in range(NH)]
        kT = resid.tile([128, S], F32R, tag="kT")
        v_sb = resid.tile([128, KT, HD], F32R, tag="v_nat")
        yT = [resid.tile([128, S], F32R, tag=f"yT{h}", name=f"yT{h}") for h in range(NH)]

        # ---------------- phase 1: projections ----------------
        def rms_row(pr_ap, t_idx, j_idx, gain_ap=None):
            sr_ap = rows.tile([1, QT], F32R, tag="sr", name=f"sr{t_idx}_{j_idx}")
            nc.scalar.activation(
                sr_ap, pr_ap, mybir.ActivationFunctionType.Sqrt,
                bias=eps_sb[0:1, 0:1], scale=1.0 / HD,
            )
            nc.vector.reciprocal(sr_ap, sr_ap)
            if gain_ap is not None:
                nc.vector.tensor_scalar_mul(sr_ap, sr_ap, gain_ap)
            nc.sync.dma_start(
                srow_d[t_idx:t_idx + 1, j_idx * QT:(j_idx + 1) * QT], sr_ap
            )

        with tc.tile_pool(name="p1", bufs=1) as p1, \
             tc.tile_pool(name="xq", bufs=2) as xqp, \
             tc.tile_pool(name="p1w", bufs=2) as p1w:
            kw_sb = p1.tile([128, KC, HD], F32R, tag="kw")
            nc.sync.dma_start(kw_sb, kw.rearrange("(c p) m -> p c m", p=128))
            vw_sb = p1.tile([128, KC, HD], F32R, tag="vw")
            nc.sync.dma_start(vw_sb, vw.rearrange("(c p) m -> p c m", p=128))

            for j in range(JT):
                xh = []
                for half in range(2):
                    xq = xqp.tile([128, KH, QT], F32R, tag="xq", name=f"xq{j}_{half}")
                    nc.sync.dma_start(
                        xq,
                        xT[half * KH * 128:(half + 1) * KH * 128,
                           j * QT:(j + 1) * QT].rearrange("(c p) t -> p c t", p=128),
                    )
                    xh.append(xq)

                def proj(w_sb, w_off, w_width, jj=j, xhh=xh):
                    pq = psA.tile([128, QT], F32, tag="acc", name=f"pacc{jj}")
                    for c in range(KC):
                        nc.tensor.matmul(
                            pq, w_sb[:, c, w_off:w_off + w_width],
                            xhh[c // KH][:, c % KH, :],
                            start=(c == 0), stop=(c == KC - 1),
                        )
                    return pq

                for h in range(NH):
                    pq = proj(qw_sb, h * HD, HD)
                    nc.scalar.copy(qT[h][:, j * QT:(j + 1) * QT], pq)
                    sq = p1w.tile([128, QT], F32R, tag="sq", name=f"sq{j}_{h}")
                    nc.scalar.square(sq, pq)
                    pr = psR.tile([1, QT], F32, tag="row", name=f"prow{j}_{h}")
                    nc.tensor.matmul(pr, ones_k, sq, start=True, stop=True)
                    rms_row(pr, h, j, gain_sb[0:1, h:h + 1])
                # k
                pk = proj(kw_sb, 0, HD)
                nc.scalar.copy(kT[:, j * QT:(j + 1) * QT], pk)
                sq = p1w.tile([128, QT], F32R, tag="sq", name=f"sqk{j}")
                nc.scalar.square(sq, pk)
                pr = psR.tile([1, QT], F32, tag="row", name=f"prowk{j}")
                nc.tensor.matmul(pr, ones_k, sq, start=True, stop=True)
                rms_row(pr, NH, j)
                # v: copy to sbuf then transpose into [token, hd] blocks
                pv = proj(vw_sb, 0, HD)
                vtmp = p1w.tile([128, QT], F32, tag="vtmp", name=f"vtmp{j}")
                nc.scalar.copy(vtmp, pv)
                for k in range(QT // 128):
                    tp = psM.tile([128, HD], F32, tag="mov", name=f"vtp{j}_{k}")
                    nc.tensor.transpose(tp, vtmp[:, k * 128:(k + 1) * 128], ident)
                    nc.vector.tensor_copy(v_sb[:, j * (QT // 128) + k, :], tp)

        # ---------------- phase 1.5: rope + rms scale ----------------
        H = RD // 2
        with tc.tile_pool(name="rope", bufs=2) as rp:
            for t in range(NH + 1):
                th = qT[t] if t < NH else kT
                tmp = rp.tile([RD, S], F32R, tag="ropetmp", name=f"rt{t}")
                nc.sync.dma_start(tmp[0:H, :], th[H:RD, :])
                nc.sync.dma_start(tmp[H:RD, :], th[0:H, :])
                m1 = rp.tile([RD, S], F32R, tag="ropem1", name=f"rm{t}")
                nc.vector.tensor_mul(m1, th[0:RD, :], cc_sb.bitcast(F32R))
                nc.vector.tensor_mul(tmp, tmp, ss_sb.bitcast(F32R))
                nc.vector.tensor_add(th[0:RD, :], m1, tmp)
                # apply rms (and gain) scale over all 128 rows
                for j in range(JT):
                    srj = rows.tile([1, QT], F32R, tag="sr", name=f"srb{t}_{j}")
                    nc.sync.dma_start(srj, srow_d[t:t + 1, j * QT:(j + 1) * QT])
                    bc = psM.tile([128, QT], F32, tag="mov", name=f"bc{t}_{j}")
                    nc.tensor.matmul(
                        bc, ones_p, srj,
                        start=True, stop=True,
                    )
                    nc.vector.tensor_mul(
                        th[:, j * QT:(j + 1) * QT], th[:, j * QT:(j + 1) * QT], bc
                    )

        # ---------------- phase 2: attention ----------------
        with tc.tile_pool(name="P", bufs=4) as ppool, \
             tc.tile_pool(name="att", bufs=3) as att:
            for h in range(NH):
                for j in range(JT):
                    ni = RPB * j + RPB  # kt tiles for this qt tile
                    yp = psA.tile([128, QT], F32, tag="acc", name=f"y{h}_{j}")
                    dp = psR.tile([1, QT], F32, tag="row", name=f"d{h}_{j}")
                    for i in range(ni):
                        sp = psM.tile([128, QT], F32, tag="mov", name=f"s{h}_{j}_{i}")
                        nc.tensor.matmul(
                            sp, kT[:, i * 128:(i + 1) * 128],
                            qT[h][:, j * QT:(j + 1) * QT],
                            start=True, stop=True,
                        )
                        P = ppool.tile([128, QT], F32R, tag="P", name=f"P{h}_{j}_{i}")
                        nc.scalar.activation(
                            P, sp, mybir.ActivationFunctionType.Exp, scale=scale
                        )
                        if i >= RPB * j:  # diagonal tile: causal mask (keep kt<=qt)
                            nc.gpsimd.affine_select(
                                P, P,
                                pattern=[[1, QT]],
                                compare_op=mybir.AluOpType.is_ge,
                                fill=0.0,
                                base=j * QT - i * 128,
                                channel_multiplier=-1,
                            )
                        nc.tensor.matmul(
                            yp, v_sb[:, i, :], P, start=(i == 0), stop=(i == ni - 1)
                        )
                        nc.tensor.matmul(
                            dp, ones_k, P, start=(i == 0), stop=(i == ni - 1)
                        )
                    rr = rows.tile([1, QT], F32R, tag="recip", name=f"r{h}_{j}")
                    nc.vector.reciprocal(rr, dp)
                    bc = psM.tile([128, QT], F32, tag="mov", name=f"ab{h}_{j}")
                    nc.tensor.matmul(bc, ones_p, rr, start=True, stop=True)
                    bcs = att.tile([128, QT], F32, tag="bcs", name=f"bs{h}_{j}")
                    nc.scalar.copy(bcs, bc)
                    nc.vector.tensor_mul(yT[h][:, j * QT:(j + 1) * QT], yp, bcs)

        # ---------------- phase 3: output projection ----------------
        ow_sb = bigw.tile([128, HC, O], F32R, tag="bigw")
        nc.sync.dma_start(ow_sb, ow.rearrange("(c p) m -> p c m", p=128))
        with tc.tile_pool(name="outp", bufs=3) as outp:
            for m in range(O // 128):
                for j in range(JT):
                    op = psA.tile([128, QT], F32, tag="acc", name=f"o{m}_{j}")
                    for h in range(NH):
                        nc.tensor.matmul(
                            op, ow_sb[:, h, m * 128:(m + 1) * 128],
                            yT[h][:, j * QT:(j + 1) * QT],
                            start=(h == 0), stop=(h == NH - 1),
                        )
                    ot = outp.tile([128, QT], F32, tag="ot", name=f"ot{m}_{j}")
                    nc.vector.tensor_copy(ot, op)
                    nc.sync.dma_start(
                        outT[m * 128:(m + 1) * 128, j * QT:(j + 1) * QT], ot
                    )
    return nc


def _rope_tables(S, RD):
    t = np.arange(S, dtype=np.float32)
    inv = (1.0 / (10000.0 ** (np.arange(0, RD, 2, dtype=np.float32) / RD))).astype(np.float32)
    f = np.outer(t, inv)  # [S, RD/2]
    cos, sin = np.cos(f).T, np.sin(f).T  # [RD/2, S]
    cc = np.concatenate([cos, cos], axis=0).astype(np.float32)
    ss = np.concatenate([sin, -sin], axis=0).astype(np.float32)
    return np.ascontiguousarray(cc), np.ascontiguousarray(ss)


_CACHED = {}


def _get_nc(S, D, NH, HD, RD, O):
    key = (S, D, NH, HD, RD, O)
    if key not in _CACHED:
        nc = bacc.Bacc("TRN2", target_bir_lowering=False, debug=False)
        build_core_kernel(nc, S, D, NH, HD, RD, O)
        nc.compile()
        _CACHED[key] = nc
    return _CACHED[key]


def make_in_maps(x, q_w, k_w, v_w, out_w, q_gain, n_kv=4, g=4, HD=128, RD=64):
    b, S, D = x.shape
    cc, ss = _rope_tables(S, RD)
    ones_k = np.ones((128, 1), np.float32)
    ones_p = np.ones((1, 128), np.float32)
    in_maps = []
    for bb in range(b):
        xT = np.ascontiguousarray(x[bb].T.astype(np.float32))
        for kv in range(n_kv):
            hs = kv * g * HD
            in_maps.append({
                "xT": xT,
                "qw": np.ascontiguousarray(q_w[hs:hs + g * HD, :].T.astype(np.float32)),
                "kw": np.ascontiguousarray(k_w[kv * HD:(kv + 1) * HD, :].T.astype(np.float32)),
                "vw": np.ascontiguousarray(v_w[kv * HD:(kv + 1) * HD, :].T.astype(np.float32)),
                "ow": np.ascontiguousarray(out_w[:, hs:hs + g * HD].T.astype(np.float32)),
                "cc": cc,
                "ss": ss,
                "gain": np.ascontiguousarray(q_gain[kv * g:(kv + 1) * g][None, :].astype(np.float32)),
                "ones_k": ones_k,
                "ones_p": ones_p,
            })
    return in_maps


def kernel(x, q_w, k_w, v_w, out_w, q_gain, _trace=False):
    x = np.asarray(x)
    b, S, D = x.shape
    n_kv, g, HD, RD = 4, 4, 128, 64
    nc = _get_nc(S, D, g, HD, RD, D)
    in_maps = make_in_maps(
        x, np.asarray(q_w), np.asarray(k_w), np.asarray(v_w),
        np.asarray(out_w), np.asarray(q_gain), n_kv, g, HD, RD,
    )
    res = run_bass_kernel_spmd(nc, in_maps, core_ids=list(range(8)), trace=_trace)
    out = np.zeros((b, S, D), dtype=np.float32)
    for bb in range(b):
        acc = np.zeros((D, S), dtype=np.float32)
        for kv in range(n_kv):
            acc += res.results[bb * n_kv + kv]["outT"]
        out[bb] = acc.T
    kernel._last_results = res
    return out


# revision 21
# speedup vs baseline: 1.1543x; 1.1543x over previous
"""Trainium2 Bass kernel for causal GQA self-attention (dense transformer block).

Sharding: 8 cores = 2 batches x 4 kv-head groups. Each core computes, for its
(batch, kv-head) pair: Q/K/V projections for its 4 q-heads + 1 kv-head,
RMS-norm + partial RoPE + q_gain, causal softmax(QK^T)V, and its partial
output projection (out_w column block). Host sums the 4 partial outputs per
batch and transposes back.

On-chip layout is "transposed": activations live as [feature, token] so every
matmul has a 512-wide moving dim (fp32r full-rate) and softmax normalization
is done with PE ones-matmuls along the kt partition axis. The main loop is
fused over 512-token qt slices: project -> rope/rms -> attention, with the
output projection trailing once the big weight buffer swaps q_w -> out_w.
"""

import sys

sys.path.insert(0, "/opt/trn_rl_repo")

import contextlib

import numpy as np

import concourse.bass as bass
import concourse.tile as tile
from concourse import bacc, mybir
from concourse.bass_utils import run_bass_kernel_spmd

F32 = mybir.dt.float32
F32R = mybir.dt.float32r

EPS = 1.1920929e-07


def build_core_kernel(nc, S, D, NH, HD, RD, O, stop_after=None):
    """Emit the per-core program. NH q-heads (each HD wide), one kv head."""
    QT = 512                      # qt tile width (moving dim)
    KC = D // 128                 # contraction chunks for projections
    KH = KC // 2                  # x chunks per half-tile
    JT = S // QT                  # qt tiles
    KT = S // 128                 # kt tiles
    RPB = KT // JT                # kt(128) tiles per qt(512) tile = 4
    HC = NH * HD // 128           # head-dim chunks of the core's q block
    NR = NH + 1                   # rms rows: NH q heads + k
    assert HD == 128 and QT == 512 and RPB == 4 and KC % 2 == 0

    dram = lambda name, shape, dt, kind="ExternalInput": nc.dram_tensor(
        name, shape, dt, kind=kind).ap()
    xT = dram("xT", [D, S], F32R)
    qw = dram("qw", [D, NH * HD], F32R)
    kvw = dram("kvw", [D, 2 * HD], F32R)          # k_w.T | v_w.T columns
    ow = dram("ow", [NH * HD, O], F32R)
    cc = dram("cc", [RD, S], F32)
    ssn = dram("ss", [RD, S], F32)
    gain5 = dram("gain5", [NR, 1], F32R)          # [g0..g3, 1.0]
    ones_kd = dram("ones_k", [128, 1], F32R)
    ones_pd = dram("ones_p", [1, 128], F32R)
    permd = dram("permT", [RD, RD], F32R)         # half-swap permutation
    e5d = dram("e5", [128, NR, 8], F32R)          # e5[:, t, m] = (m == t)
    maskd = dram("masks", [128, 128], F32R)       # upper-tri causal stripe mask
    identd = dram("ident", [128, 128], F32)
    outT = dram("outT", [O, S], F32, "ExternalOutput")

    ascale = 1.0 / float(np.sqrt(HD))

    with tile.TileContext(nc) as tc, nc.allow_low_precision(
        reason="float32r tiles are 4-byte fp32"
    ), contextlib.ExitStack() as ctx:
        consts = ctx.enter_context(tc.tile_pool(name="consts", bufs=1))
        resid = ctx.enter_context(tc.tile_pool(name="resid", bufs=1))
        bigw = ctx.enter_context(tc.tile_pool(name="bigw", bufs=1))
        rows = ctx.enter_context(tc.tile_pool(name="rows", bufs=3))
        ppool = ctx.enter_context(tc.tile_pool(name="P", bufs=6))
        work = ctx.enter_context(tc.tile_pool(name="work", bufs=2))
        psA = ctx.enter_context(tc.tile_pool(name="psA", bufs=2, space="PSUM"))
        psM = ctx.enter_context(tc.tile_pool(name="psM", bufs=4, space="PSUM"))
        psR = ctx.enter_context(tc.tile_pool(name="psR", bufs=2, space="PSUM"))

        # constants
        ones_k = consts.tile([128, 1], F32R, tag="ones_k")
        ones_p = consts.tile([1, 128], F32R, tag="ones_p")
        e5 = consts.tile([128, NR, 8], F32R, tag="e5")
        permT = consts.tile([RD, RD], F32R, tag="permT")
        masks = consts.tile([128, 128], F32R, tag="masks")
        ident = consts.tile([128, 128], F32, tag="ident")
        cc_sb = consts.tile([RD, S], F32R, tag="cc")
        ss_sb = consts.tile([RD, S], F32R, tag="ss")
        gain_sb = consts.tile([NR, 1], F32R, tag="gain")
        eps_sb = consts.tile([NR, 1], F32, tag="eps")
        nc.vector.memset(eps_sb, EPS)

        # big weight buffer: q_w during the main loop, out_w afterwards
        qw_sb = bigw.tile([128, KC, NH * HD], F32R, tag="bigw")
        kvw_sb = resid.tile([128, KC, 2 * HD], F32R, tag="kvw")

        # activation residents: [feature 128, token S]; qT doubles as yT
        qT = [resid.tile([128, S], F32R, tag=f"qT{h}", name=f"qT{h}") for h in range(NH)]
        kT = resid.tile([128, S], F32R, tag="kT")
        v_sb = resid.tile([128, KT, HD], F32R, tag="v_nat")

        with tc.tile_pool(name="xq", bufs=4) as xqp:
            for j in range(JT):
                jsl = slice(j * QT, (j + 1) * QT)
                KQ = KC // 4
                xh = []
                for quar in range(4):
                    xq = xqp.tile([128, KQ, QT], F32R, tag="xq", name=f"xq{j}_{quar}")
                    nc.sync.dma_start(
                        xq,
                        xT[quar * KQ * 128:(quar + 1) * KQ * 128,
                           jsl].rearrange("(c p) t -> p c t", p=128),
                    )
                    xh.append(xq)
                if j == 0:
                    qwr = qw.rearrange("(c p) m -> p c m", p=128)
                    for qc in range(4):
                        nc.sync.dma_start(
                            qw_sb[:, qc * (KC // 4):(qc + 1) * (KC // 4), :],
                            qwr[:, qc * (KC // 4):(qc + 1) * (KC // 4), :],
                        )
                    nc.sync.dma_start(kvw_sb, kvw.rearrange("(c p) m -> p c m", p=128))
                    nc.sync.dma_start(ones_k, ones_kd)
                    nc.sync.dma_start(ones_p, ones_pd)
                    nc.sync.dma_start(e5, e5d)
                    nc.sync.dma_start(ident, identd)
                    nc.sync.dma_start(cc_sb, cc.bitcast(F32R))
                    nc.sync.dma_start(ss_sb, ssn.bitcast(F32R))
                    nc.sync.dma_start(gain_sb, gain5)
                    nc.sync.dma_start(permT, permd)
                    nc.sync.dma_start(masks, maskd)

                def proj(w_sb, w_off, jj=j, xhh=xh, KQ=KQ):
                    pq = psA.tile([128, QT], F32, tag="acc", name=f"pacc{jj}_{w_off}")
                    for c in range(KC):
                        nc.tensor.matmul(
                            pq, w_sb[:, c, w_off:w_off + HD],
                            xhh[c // KQ][:, c % KQ, :],
                            start=(c == 0), stop=(c == KC - 1),
                        )
                    return pq

                # --- projections + squared-sum rows ---
                sr8 = psR.tile([8, QT], F32, tag="row8", name=f"sr8_{j}")
                for t in range(NR):          # q0..q3, then k
                    if t < NH:
                        pq = proj(qw_sb, t * HD)
                        dst = qT[t][:, jsl]
                    else:
                        pq = proj(kvw_sb, 0)
                        dst = kT[:, jsl]
                    nc.vector.tensor_copy(dst, pq)
                    sq = work.tile([128, QT], F32R, tag="sq", name=f"sq{j}_{t}")
                    nc.vector.tensor_mul(sq, dst, dst)
                    nc.tensor.matmul(sr8, e5[:, t, :], sq, start=(t == 0), stop=(t == NR - 1))
                # v
                pv = proj(kvw_sb, HD)
                vtmp = work.tile([128, QT], F32, tag="vtmp", name=f"vtmp{j}")
                nc.vector.tensor_copy(vtmp, pv)
                for k in range(RPB):
                    tp = psM.tile([128, HD], F32, tag="mov", name=f"vtp{j}_{k}")
                    nc.tensor.transpose(tp, vtmp[:, k * 128:(k + 1) * 128], ident)
                    nc.vector.tensor_copy(v_sb[:, j * RPB + k, :], tp)

                # --- rms scale rows: gain / sqrt(ms + eps) ---
                srow5 = rows.tile([NR, QT], F32R, tag="srow5", name=f"srow5_{j}")
                nc.scalar.activation(
                    srow5, sr8[0:NR, :], mybir.ActivationFunctionType.Sqrt,
                    bias=eps_sb, scale=1.0 / HD,
                )
                nc.vector.reciprocal(srow5, srow5)
                nc.vector.tensor_scalar_mul(srow5, srow5, gain_sb.bitcast(F32))
                srt = []
                for t in range(NR):
                    s1 = rows.tile([1, QT], F32R, tag="srt", name=f"srt{j}_{t}")
                    nc.sync.dma_start(s1, srow5[t:t + 1, :])
                    srt.append(s1)

                # --- rope + scale per row group ---
                for t in range(NR):
                    th = qT[t] if t < NH else kT
                    tmp = psM.tile([RD, QT], F32, tag="mov", name=f"rope{j}_{t}")
                    nc.tensor.matmul(tmp, permT, th[0:RD, jsl], start=True, stop=True)
                    t2 = work.tile([RD, QT], F32R, tag="ropet2", name=f"t2_{j}_{t}")
                    nc.vector.tensor_mul(t2, tmp, ss_sb[:, jsl])
                    m1 = work.tile([RD, QT], F32R, tag="ropem1", name=f"m1_{j}_{t}")
                    nc.vector.tensor_mul(m1, th[0:RD, jsl], cc_sb[:, jsl])
                    nc.vector.tensor_add(th[0:RD, jsl], m1, t2)
                    bc = psM.tile([128, QT], F32, tag="mov", name=f"bc{j}_{t}")
                    nc.tensor.matmul(bc, ones_p, srt[t], start=True, stop=True)
                    nc.vector.tensor_mul(th[:, jsl], th[:, jsl], bc)

                # --- attention for all heads on this qt slice ---
                ni = RPB * j + RPB
                for h in range(NH):
                    yp = psA.tile([128, QT], F32, tag="acc", name=f"y{h}_{j}")
                    dp = psR.tile([8, QT], F32, tag="row8", name=f"d{h}_{j}")
                    for i in range(ni):
                        sp = psM.tile([128, QT], F32, tag="mov", name=f"s{h}_{j}_{i}")
                        nc.tensor.matmul(
                            sp, kT[:, i * 128:(i + 1) * 128], qT[h][:, jsl],
                            start=True, stop=True,
                        )
                        P = ppool.tile([128, QT], F32R, tag="P", name=f"P{h}_{j}_{i}")
                        nc.scalar.activation(
                            P, sp, mybir.ActivationFunctionType.Exp, scale=ascale
                        )
                        if i >= RPB * j:  # diagonal tile: causal mask
                            d = i - RPB * j
                            nc.vector.tensor_mul(
                                P[:, d * 128:(d + 1) * 128],
                                P[:, d * 128:(d + 1) * 128], masks)
                            if d > 0:
                                nc.vector.tensor_scalar_mul(P[:, 0:d * 128], P[:, 0:d * 128], 0.0)
                        nc.tensor.matmul(
                            yp, v_sb[:, i, :], P, start=(i == 0), stop=(i == ni - 1)
                        )
                        nc.tensor.matmul(
                            dp[0:1, :], ones_k, P, start=(i == 0), stop=(i == ni - 1)
                        )
                    rr = rows.tile([1, QT], F32R, tag="recip", name=f"r{h}_{j}")
                    nc.vector.reciprocal(rr, dp[0:1, :])
                    bc = psM.tile([128, QT], F32, tag="mov", name=f"ab{h}_{j}")
                    nc.tensor.matmul(bc, ones_p, rr, start=True, stop=True)
                    bcs = work.tile([128, QT], F32, tag="bcs", name=f"bs{h}_{j}")
                    nc.vector.tensor_copy(bcs, bc)
                    # overwrite qT[h] slice with attention output (yT alias)
                    nc.vector.tensor_mul(qT[h][:, jsl], yp, bcs)

        if stop_after == "attn":
            return nc

        # ---------------- trailing phase: output projection ----------------
        ow_sb = bigw.tile([128, HC, O], F32R, tag="bigw")
        owr = ow.rearrange("(c p) m -> p c m", p=128)
        for oc in range(4):
            osl = slice(oc * (O // 4), (oc + 1) * (O // 4))
            nc.sync.dma_start(ow_sb[:, :, osl], owr[:, :, osl])
        with tc.tile_pool(name="outp", bufs=3) as outp:
            for m in range(O // 128):
                orow = outp.tile([128, S], F32, tag="orow", name=f"orow{m}")
                for j in range(JT):
                    op = psA.tile([128, QT], F32, tag="acc", name=f"o{m}_{j}")
                    for h in range(NH):
                        nc.tensor.matmul(
                            op, ow_sb[:, h, m * 128:(m + 1) * 128],
                            qT[h][:, j * QT:(j + 1) * QT],
                            start=(h == 0), stop=(h == NH - 1),
                        )
                    nc.vector.tensor_copy(orow[:, j * QT:(j + 1) * QT], op)
                nc.sync.dma_start(outT[m * 128:(m + 1) * 128, :], orow)
    return nc


def _rope_tables(S, RD):
    t = np.arange(S, dtype=np.float32)
    inv = (1.0 / (10000.0 ** (np.arange(0, RD, 2, dtype=np.float32) / RD))).astype(np.float32)
    f = np.outer(t, inv)  # [S, RD/2]
    cos, sin = np.cos(f).T, np.sin(f).T  # [RD/2, S]
    cc = np.concatenate([cos, cos], axis=0).astype(np.float32)
    ss = np.concatenate([sin, -sin], axis=0).astype(np.float32)
    return np.ascontiguousarray(cc), np.ascontiguousarray(ss)


def _const_inputs(NH, HD, RD, RPB=4, QT=512):
    NR = NH + 1
    h = RD // 2
    perm = np.zeros((RD, RD), np.float32)
    perm[np.arange(h), np.arange(h) + h] = 1.0
    perm[np.arange(h) + h, np.arange(h)] = 1.0
    e5 = np.zeros((128, NR, 8), np.float32)
    for t in range(NR):
        e5[:, t, t] = 1.0
    masks = np.triu(np.ones((128, 128), np.float32))
    ident = np.eye(128, dtype=np.float32)
    return {
        "permT": perm,  # symmetric swap -> its own transpose
        "e5": e5,
        "masks": np.ascontiguousarray(masks),
        "ident": ident,
        "ones_k": np.ones((128, 1), np.float32),
        "ones_p": np.ones((1, 128), np.float32),
    }


_CACHED = {}


def _get_nc(S, D, NH, HD, RD, O):
    key = (S, D, NH, HD, RD, O)
    if key not in _CACHED:
        nc = bacc.Bacc("TRN2", target_bir_lowering=False, debug=False)
        build_core_kernel(nc, S, D, NH, HD, RD, O)
        nc.compile()
        _CACHED[key] = nc
    return _CACHED[key]


def make_in_maps(x, q_w, k_w, v_w, out_w, q_gain, n_kv=4, g=4, HD=128, RD=64):
    b, S, D = x.shape
    cc, ss = _rope_tables(S, RD)
    consts = _const_inputs(g, HD, RD)
    in_maps = []
    for bb in range(b):
        xT = np.ascontiguousarray(x[bb].T.astype(np.float32))
        for kv in range(n_kv):
            hs = kv * g * HD
            kvw = np.concatenate(
                [k_w[kv * HD:(kv + 1) * HD, :].T, v_w[kv * HD:(kv + 1) * HD, :].T],
                axis=1,
            ).astype(np.float32)
            gain5 = np.concatenate(
                [np.asarray(q_gain[kv * g:(kv + 1) * g], np.float32), [1.0]]
            ).reshape(g + 1, 1).astype(np.float32)
            in_maps.append({
                "xT": xT,
                "qw": np.ascontiguousarray(q_w[hs:hs + g * HD, :].T.astype(np.float32)),
                "kvw": np.ascontiguousarray(kvw),
                "ow": np.ascontiguousarray(out_w[:, hs:hs + g * HD].T.astype(np.float32)),
                "cc": cc,
                "ss": ss,
                "gain5": gain5,
                **consts,
            })
    return in_maps


def kernel(x, q_w, k_w, v_w, out_w, q_gain, _trace=False):
    x = np.asarray(x)
    b, S, D = x.shape
    n_kv, g, HD, RD = 4, 4, 128, 64
    nc = _get_nc(S, D, g, HD, RD, D)
    in_maps = make_in_maps(
        x, np.asarray(q_w), np.asarray(k_w), np.asarray(v_w),
        np.asarray(out_w), np.asarray(q_gain), n_kv, g, HD, RD,
    )
    res = run_bass_kernel_spmd(nc, in_maps, core_ids=list(range(8)), trace=_trace)
    out = np.zeros((b, S, D), dtype=np.float32)
    for bb in range(b):
        acc = np.zeros((D, S), dtype=np.float32)
        for kv in range(n_kv):
            acc += res.results[bb * n_kv + kv]["outT"]
        out[bb] = acc.T
    kernel._last_results = res
    return out


# revision 24
# speedup vs baseline: 38.1685x; 33.0672x over previous
"""Trainium2 Bass kernel for causal GQA self-attention (dense transformer block).

Sharding: 8 cores = 2 batches x 4 kv-head groups. Each core computes, for its
(batch, kv-head) pair: Q/K/V projections for its 4 q-heads + 1 kv-head,
RMS-norm + partial RoPE + q_gain, causal softmax(QK^T)V, and its partial
output projection (out_w column block). Host sums the 4 partial outputs per
batch and transposes back.

On-chip layout is "transposed": activations live as [feature, token] so every
matmul has a 512-wide moving dim (fp32r full-rate) and softmax normalization
is done with PE ones-matmuls along the kt partition axis. The main loop is
fused over 512-token qt slices: project -> rope/rms -> attention, with the
output projection trailing once the big weight buffer swaps q_w -> out_w.
"""

import sys

sys.path.insert(0, "/opt/trn_rl_repo")

import contextlib

import numpy as np

import concourse.bass as bass
import concourse.tile as tile
from concourse import bacc, mybir
from concourse.bass_utils import run_bass_kernel_spmd

F32 = mybir.dt.float32
F32R = mybir.dt.float32r

EPS = 1.1920929e-07


def build_core_kernel(nc, S, D, NH, HD, RD, O, stop_after=None):
    """Emit the per-core program. NH q-heads (each HD wide), one kv head."""
    QT = 512                      # qt tile width (moving dim)
    KC = D // 128                 # contraction chunks for projections
    KH = KC // 2                  # x chunks per half-tile
    JT = S // QT                  # qt tiles
    KT = S // 128                 # kt tiles
    RPB = KT // JT                # kt(128) tiles per qt(512) tile = 4
    HC = NH * HD // 128           # head-dim chunks of the core's q block
    NR = NH + 1                   # rms rows: NH q heads + k
    assert HD == 128 and QT == 512 and RPB == 4 and KC % 2 == 0

    dram = lambda name, shape, dt, kind="ExternalInput": nc.dram_tensor(
        name, shape, dt, kind=kind).ap()
    xT = dram("xT", [D, S], F32R)
    qw = dram("qw", [D, NH * HD], F32R)
    kvw = dram("kvw", [D, 2 * HD], F32R)          # k_w.T | v_w.T columns
    ow = dram("ow", [NH * HD, O], F32R)
    cc = dram("cc", [RD, S], F32)
    ssn = dram("ss", [RD, S], F32)
    gain5 = dram("gain5", [NR, 1], F32R)          # [g0..g3, 1.0]
    ones_kd = dram("ones_k", [128, 1], F32R)
    ones_pd = dram("ones_p", [1, 128], F32R)
    permd = dram("permT", [RD, RD], F32R)         # half-swap permutation
    e5d = dram("e5", [128, NR, 8], F32R)          # e5[:, t, m] = (m == t)
    maskd = dram("masks", [128, 128], F32R)       # upper-tri causal stripe mask
    identd = dram("ident", [128, 128], F32)
    outT = dram("outT", [O, S], F32, "ExternalOutput")

    ascale = 1.0 / float(np.sqrt(HD))

    with tile.TileContext(nc) as tc, nc.allow_low_precision(
        reason="float32r tiles are 4-byte fp32"
    ), contextlib.ExitStack() as ctx:
        consts = ctx.enter_context(tc.tile_pool(name="consts", bufs=1))
        resid = ctx.enter_context(tc.tile_pool(name="resid", bufs=1))
        bigw = ctx.enter_context(tc.tile_pool(name="bigw", bufs=1))
        rows = ctx.enter_context(tc.tile_pool(name="rows", bufs=3))
        ppool = ctx.enter_context(tc.tile_pool(name="P", bufs=6))
        work = ctx.enter_context(tc.tile_pool(name="work", bufs=2))
        psA = ctx.enter_context(tc.tile_pool(name="psA", bufs=2, space="PSUM"))
        psM = ctx.enter_context(tc.tile_pool(name="psM", bufs=4, space="PSUM"))
        psR = ctx.enter_context(tc.tile_pool(name="psR", bufs=2, space="PSUM"))

        # constants
        ones_k = consts.tile([128, 1], F32R, tag="ones_k")
        ones_p = consts.tile([1, 128], F32R, tag="ones_p")
        e5 = consts.tile([128, NR, 8], F32R, tag="e5")
        permT = consts.tile([RD, RD], F32R, tag="permT")
        masks = consts.tile([128, 128], F32R, tag="masks")
        ident = consts.tile([128, 128], F32, tag="ident")
        cc_sb = consts.tile([RD, S], F32R, tag="cc")
        ss_sb = consts.tile([RD, S], F32R, tag="ss")
        gain_sb = consts.tile([NR, 1], F32R, tag="gain")
        eps_sb = consts.tile([NR, 1], F32, tag="eps")
        nc.vector.memset(eps_sb, EPS)

        # big weight buffer: q_w during the main loop, out_w afterwards
        qw_sb = bigw.tile([128, KC, NH * HD], F32R, tag="bigw")
        kvw_sb = resid.tile([128, KC, 2 * HD], F32R, tag="kvw")

        # activation residents: [feature 128, token S]; qT doubles as yT
        qT = [resid.tile([128, S], F32R, tag=f"qT{h}", name=f"qT{h}") for h in range(NH)]
        kT = resid.tile([128, S], F32R, tag="kT")
        v_sb = resid.tile([128, KT, HD], F32R, tag="v_nat")

        with tc.tile_pool(name="xq", bufs=4) as xqp:
            for j in range(JT):
                jsl = slice(j * QT, (j + 1) * QT)
                KQ = KC // 4
                xh = []
                for quar in range(4):
                    xq = xqp.tile([128, KQ, QT], F32R, tag="xq", name=f"xq{j}_{quar}")
                    nc.sync.dma_start(
                        xq,
                        xT[quar * KQ * 128:(quar + 1) * KQ * 128,
                           jsl].rearrange("(c p) t -> p c t", p=128),
                    )
                    xh.append(xq)
                if j == 0:
                    qwr = qw.rearrange("(c p) m -> p c m", p=128)
                    for qc in range(4):
                        nc.sync.dma_start(
                            qw_sb[:, qc * (KC // 4):(qc + 1) * (KC // 4), :],
                            qwr[:, qc * (KC // 4):(qc + 1) * (KC // 4), :],
                        )
                    nc.sync.dma_start(kvw_sb, kvw.rearrange("(c p) m -> p c m", p=128))
                    nc.sync.dma_start(ones_k, ones_kd)
                    nc.sync.dma_start(ones_p, ones_pd)
                    nc.sync.dma_start(e5, e5d)
                    nc.sync.dma_start(ident, identd)
                    nc.sync.dma_start(cc_sb, cc.bitcast(F32R))
                    nc.sync.dma_start(ss_sb, ssn.bitcast(F32R))
                    nc.sync.dma_start(gain_sb, gain5)
                    nc.sync.dma_start(permT, permd)
                    nc.sync.dma_start(masks, maskd)

                def proj(w_sb, w_off, jj=j, xhh=xh, KQ=KQ):
                    pq = psA.tile([128, QT], F32, tag="acc", name=f"pacc{jj}_{w_off}")
                    for c in range(KC):
                        nc.tensor.matmul(
                            pq, w_sb[:, c, w_off:w_off + HD],
                            xhh[c // KQ][:, c % KQ, :],
                            start=(c == 0), stop=(c == KC - 1),
                        )
                    return pq

                # --- projections + squared-sum rows ---
                sr8 = psR.tile([8, QT], F32, tag="row8", name=f"sr8_{j}")
                for t in range(NR):          # q0..q3, then k
                    if t < NH:
                        pq = proj(qw_sb, t * HD)
                        dst = qT[t][:, jsl]
                    else:
                        pq = proj(kvw_sb, 0)
                        dst = kT[:, jsl]
                    nc.vector.tensor_copy(dst, pq)
                    sq = work.tile([128, QT], F32R, tag="sq", name=f"sq{j}_{t}")
                    nc.vector.tensor_mul(sq, dst, dst)
                    nc.tensor.matmul(sr8, e5[:, t, :], sq, start=(t == 0), stop=(t == NR - 1))
                # v
                pv = proj(kvw_sb, HD)
                vtmp = work.tile([128, QT], F32, tag="vtmp", name=f"vtmp{j}")
                nc.vector.tensor_copy(vtmp, pv)
                for k in range(RPB):
                    tp = psM.tile([128, HD], F32, tag="mov", name=f"vtp{j}_{k}")
                    nc.tensor.transpose(tp, vtmp[:, k * 128:(k + 1) * 128], ident)
                    nc.vector.tensor_copy(v_sb[:, j * RPB + k, :], tp)

                # --- rms scale rows: gain / sqrt(ms + eps) ---
                srow5 = rows.tile([NR, QT], F32R, tag="srow5", name=f"srow5_{j}")
                nc.scalar.activation(
                    srow5, sr8[0:NR, :], mybir.ActivationFunctionType.Sqrt,
                    bias=eps_sb, scale=1.0 / HD,
                )
                nc.vector.reciprocal(srow5, srow5)
                nc.vector.tensor_scalar_mul(srow5, srow5, gain_sb.bitcast(F32))
                srt = []
                for t in range(NR):
                    s1 = rows.tile([1, QT], F32R, tag="srt", name=f"srt{j}_{t}")
                    nc.sync.dma_start(s1, srow5[t:t + 1, :])
                    srt.append(s1)

                # --- rope + scale per row group ---
                for t in range(NR):
                    th = qT[t] if t < NH else kT
                    tmp = psM.tile([RD, QT], F32, tag="mov", name=f"rope{j}_{t}")
                    nc.tensor.matmul(tmp, permT, th[0:RD, jsl], start=True, stop=True)
                    t2 = work.tile([RD, QT], F32R, tag="ropet2", name=f"t2_{j}_{t}")
                    nc.vector.tensor_mul(t2, tmp, ss_sb[:, jsl])
                    m1 = work.tile([RD, QT], F32R, tag="ropem1", name=f"m1_{j}_{t}")
                    nc.vector.tensor_mul(m1, th[0:RD, jsl], cc_sb[:, jsl])
                    nc.vector.tensor_add(th[0:RD, jsl], m1, t2)
                    bc = psM.tile([128, QT], F32, tag="mov", name=f"bc{j}_{t}")
                    nc.tensor.matmul(bc, ones_p, srt[t], start=True, stop=True)
                    nc.vector.tensor_mul(th[:, jsl], th[:, jsl], bc)

                # --- attention for all heads on this qt slice ---
                ni = RPB * j + RPB
                for h in range(NH):
                    yp = psA.tile([128, QT], F32, tag="acc", name=f"y{h}_{j}")
                    dp = psR.tile([8, QT], F32, tag="row8", name=f"d{h}_{j}")
                    for i in range(ni):
                        sp = psM.tile([128, QT], F32, tag="mov", name=f"s{h}_{j}_{i}")
                        nc.tensor.matmul(
                            sp, kT[:, i * 128:(i + 1) * 128], qT[h][:, jsl],
                            start=True, stop=True,
                        )
                        P = ppool.tile([128, QT], F32R, tag="P", name=f"P{h}_{j}_{i}")
                        nc.scalar.activation(
                            P, sp, mybir.ActivationFunctionType.Exp, scale=ascale
                        )
                        if i >= RPB * j:  # diagonal tile: causal mask
                            d = i - RPB * j
                            nc.vector.tensor_mul(
                                P[:, d * 128:(d + 1) * 128],
                                P[:, d * 128:(d + 1) * 128], masks)
                            if d > 0:
                                nc.vector.tensor_scalar_mul(P[:, 0:d * 128], P[:, 0:d * 128], 0.0)
                        nc.tensor.matmul(
                            yp, v_sb[:, i, :], P, start=(i == 0), stop=(i == ni - 1)
                        )
                        nc.tensor.matmul(
                            dp[0:1, :], ones_k, P, start=(i == 0), stop=(i == ni - 1)
                        )
                    rr = rows.tile([1, QT], F32R, tag="recip", name=f"r{h}_{j}")
                    nc.vector.reciprocal(rr, dp[0:1, :])
                    bc = psM.tile([128, QT], F32, tag="mov", name=f"ab{h}_{j}")
                    nc.tensor.matmul(bc, ones_p, rr, start=True, stop=True)
                    bcs = work.tile([128, QT], F32, tag="bcs", name=f"bs{h}_{j}")
                    nc.vector.tensor_copy(bcs, bc)
                    # overwrite qT[h] slice with attention output (yT alias)
                    nc.vector.tensor_mul(qT[h][:, jsl], yp, bcs)

        if stop_after == "attn":
            return nc

        # ---------------- trailing phase: output projection ----------------
        ow_sb = bigw.tile([128, HC, O], F32R, tag="bigw")
        owr = ow.rearrange("(c p) m -> p c m", p=128)
        for oc in range(4):
            osl = slice(oc * (O // 4), (oc + 1) * (O // 4))
            nc.sync.dma_start(ow_sb[:, :, osl], owr[:, :, osl])
        with tc.tile_pool(name="outp", bufs=3) as outp:
            for m in range(O // 128):
                orow = outp.tile([128, S], F32, tag="orow", name=f"orow{m}")
                for j in range(JT):
                    op = psA.tile([128, QT], F32, tag="acc", name=f"o{m}_{j}")
                    for h in range(NH):
                        nc.tensor.matmul(
                            op, ow_sb[:, h, m * 128:(m + 1) * 128],
                            qT[h][:, j * QT:(j + 1) * QT],
                            start=(h == 0), stop=(h == NH - 1),
                        )
                    nc.vector.tensor_copy(orow[:, j * QT:(j + 1) * QT], op)
                nc.sync.dma_start(outT[m * 128:(m + 1) * 128, :], orow)
    return nc


def _rope_tables(S, RD):
    t = np.arange(S, dtype=np.float32)
    inv = (1.0 / (10000.0 ** (np.arange(0, RD, 2, dtype=np.float32) / RD))).astype(np.float32)
    f = np.outer(t, inv)  # [S, RD/2]
    cos, sin = np.cos(f).T, np.sin(f).T  # [RD/2, S]
    cc = np.concatenate([cos, cos], axis=0).astype(np.float32)
    ss = np.concatenate([sin, -sin], axis=0).astype(np.float32)
    return np.ascontiguousarray(cc), np.ascontiguousarray(ss)


def _const_inputs(NH, HD, RD, RPB=4, QT=512):
    NR = NH + 1
    h = RD // 2
    perm = np.zeros((RD, RD), np.float32)
    perm[np.arange(h), np.arange(h) + h] = 1.0
    perm[np.arange(h) + h, np.arange(h)] = 1.0
    e5 = np.zeros((128, NR, 8), np.float32)
    for t in range(NR):
        e5[:, t, t] = 1.0
    masks = np.triu(np.ones((128, 128), np.float32))
    ident = np.eye(128, dtype=np.float32)
    return {
        "permT": perm,  # symmetric swap -> its own transpose
        "e5": e5,
        "masks": np.ascontiguousarray(masks),
        "ident": ident,
        "ones_k": np.ones((128, 1), np.float32),
        "ones_p": np.ones((1, 128), np.float32),
    }


_CACHED = {}


def _get_nc(S, D, NH, HD, RD, O):
    key = (S, D, NH, HD, RD, O)
    if key not in _CACHED:
        nc = bacc.Bacc("TRN2", target_bir_lowering=False, debug=False)
        build_core_kernel(nc, S, D, NH, HD, RD, O)
        nc.compile()
        _CACHED[key] = nc
    return _CACHED[key]


def make_in_maps(x, q_w, k_w, v_w, out_w, q_gain, n_kv=4, g=4, HD=128, RD=64):
    b, S, D = x.shape
    cc, ss = _rope_tables(S, RD)
    consts = _const_inputs(g, HD, RD)
    in_maps = []
    for bb in range(b):
        xT = np.ascontiguousarray(x[bb].T.astype(np.float32))
        for kv in range(n_kv):
            hs = kv * g * HD
            kvw = np.concatenate(
                [k_w[kv * HD:(kv + 1) * HD, :].T, v_w[kv * HD:(kv + 1) * HD, :].T],
                axis=1,
            ).astype(np.float32)
            gain5 = np.concatenate(
                [np.asarray(q_gain[kv * g:(kv + 1) * g], np.float32), [1.0]]
            ).reshape(g + 1, 1).astype(np.float32)
            in_maps.append({
                "xT": xT,
                "qw": np.ascontiguousarray(q_w[hs:hs + g * HD, :].T.astype(np.float32)),
                "kvw": np.ascontiguousarray(kvw),
                "ow": np.ascontiguousarray(out_w[:, hs:hs + g * HD].T.astype(np.float32)),
                "cc": cc,
                "ss": ss,
                "gain5": gain5,
                **consts,
            })
    return in_maps


def kernel(x, q_w, k_w, v_w, out_w, q_gain, _trace=False):
    x = np.asarray(x)
    b, S, D = x.shape
    n_kv, g, HD, RD = 4, 4, 128, 64
    nc = _get_nc(S, D, g, HD, RD, D)
    in_maps = make_in_maps(
        x, np.asarray(q_w), np.asarray(k_w), np.asarray(v_w),
        np.asarray(out_w), np.asarray(q_gain), n_kv, g, HD, RD,
    )
    res = run_bass_kernel_spmd(nc, in_maps, core_ids=list(range(8)), trace=_trace)
    out = np.zeros((b, S, D), dtype=np.float32)
    for bb in range(b):
        acc = np.zeros((D, S), dtype=np.float32)
        for kv in range(n_kv):
            acc += res.results[bb * n_kv + kv]["outT"]
        out[bb] = acc.T
    kernel._last_results = res
    return out
